# revision 8
# baseline (speedup 1.0000x reference)
"""Trainium2 Bass kernel for the sparse-attention nn module (nn_BDH_48421461295735).

Strategy: 8 NeuronCores = 8 (batch, head) pairs (B=2 x NH=4).  Each core runs
all 4 layers for its head; the only cross-core traffic is a per-layer
AllReduce (within each batch's group of 4 cores) of the per-head decoder
partial p = (x_sparse*y_sparse) @ dec_h, chunked by 512 token rows (bf16) so
it pipelines behind compute.

The work is one global software pipeline over chunk iterations
g = (layer, ts) of 512 token columns:
  - score units (chunk g) interleaved with p1 units (chunk g+1):
    p1 unit j: x_sparseT = relu(enc^T xT) (scalar relu), rope via 6
    tensor_tensor ops (DVE for j<5, gpsimd for j>=5) -> qrT cols.
    score unit sb: causal score tile qrT^T qrT -> PSUM, strict mask on the
    diagonal (DVE), PSUM->SBUF copy on the scalar engine, ykv accumulation
    lagged one unit.
  - ykv LN (scalar-engine heavy: Square+accum, Sqrt, Identity with [P,1]
    scale/bias), then p3 x_sparse recompute (relus alternating DVE/scalar,
    results held in a 16-deep pool), ykv transposes, then y_sparse / xy / p
    with ys-matmuls emitted 3 k-steps ahead of the p matmuls.
  - AR: DMA p rows -> DRAM bf16, 4-core AllReduce chunk.
  - p4 for chunk g-2 (defer gives the AllReduce ~2 chunks of compute to
    hide): x' = ln(x + p_sum) -> x ping-pong buffers + xT transposes.
x_bf / xT are double-buffered by layer parity so p4 can run while later
chunks of the same layer still read the old x.

The neuron axis is host-permuted (evens then odds) so the rope pair partner
of partition-tile j is partition-tile j+nJ (no cross-partition shuffles).
"""

import math
import sys

import numpy as np

for _p in ("/opt/trn_rl_repo",):
    if _p not in sys.path:
        sys.path.insert(0, _p)

import concourse.bass as bass
import concourse.bacc as bacc
import concourse.mybir as mybir
import concourse.tile as tile
from concourse.bass_utils import run_bass_kernel_spmd

F32 = mybir.dt.float32
BF16 = mybir.dt.bfloat16
AF = mybir.ActivationFunctionType
ALU = mybir.AluOpType

FULL_CFG = dict(T=2048, D=256, N=2048, NL=4, V=256, NH=4, B=2)
P = 128
SUP = 512
EPS = 1e-5


def build_nc(cfg, mm_dt=BF16, n_cores=8):
    T, D, N, NL, V = cfg["T"], cfg["D"], cfg["N"], cfg["NL"], cfg["V"]
    NH = cfg["NH"]
    assert T % SUP == 0 and D % P == 0 and N % 256 == 0 and V == D
    nTB, nTS, nD, nK = T // P, T // SUP, D // P, N // P
    nJ = nK // 2
    nQ = SUP // P  # 4
    nG = NL * nTS
    # p4 deferral in chunk iterations; the cross-layer p1 prefetch of chunk
    # (l+1, 0) at iteration (l, nTS-1) reads xT written by p4(l, 0), which
    # is emitted at the end of iteration (l, DEFER) -- so DEFER <= nTS-2.
    XLAYER_PREFETCH = nTS >= 2
    DEFER = max(0, min(2, nTS - 2)) if XLAYER_PREFETCH else 0
    ROPE_DVE_J = max(1, (nJ * 5 + 7) // 8)  # j < this -> DVE, else gpsimd

    nc = bacc.Bacc("TRN2", target_bir_lowering=False, debug=False,
                   num_devices=n_cores)

    x0_d = nc.dram_tensor("x0", [T, D], F32, kind="ExternalInput")
    wenc_d = nc.dram_tensor("wenc", [D, N], mm_dt, kind="ExternalInput")
    wencv_d = nc.dram_tensor("wencv", [D, N], mm_dt, kind="ExternalInput")
    wdec_d = nc.dram_tensor("wdec", [N, D], mm_dt, kind="ExternalInput")
    wlm_d = nc.dram_tensor("wlm", [D, V], mm_dt, kind="ExternalInput")
    ctab_d = nc.dram_tensor("ctab", [N // 2, T], mm_dt, kind="ExternalInput")
    stab_d = nc.dram_tensor("stab", [N // 2, T], mm_dt, kind="ExternalInput")
    ident_d = nc.dram_tensor("ident", [P, P], mm_dt, kind="ExternalInput")
    maskt_d = nc.dram_tensor("maskt", [P, P], mm_dt, kind="ExternalInput")
    out_d = nc.dram_tensor("out", [T, V], F32, kind="ExternalOutput")

    # AllReduce groups: one group of NH cores per batch.
    RG = [list(range(g * NH, (g + 1) * NH)) for g in range(n_cores // NH)]

    with tile.TileContext(nc) as tc:
        _keep = []  # keep tc.tile free-closures alive (GC would release pools)

        def ptile(shape, dtype, name, **kw):
            t, free = tc.tile(shape, dtype, name=name, **kw)
            _keep.append(free)
            return t

        # ---- persistent SBUF tensors ----
        wenc_sb = [ptile([P, N], mm_dt, name=f"wenc{d}") for d in range(nD)]
        wencv_sb = [ptile([P, N], mm_dt, name=f"wencv{d}") for d in range(nD)]
        wdec_sb = [ptile([P, D], mm_dt, name=f"wdec{k}") for k in range(nK)]
        wlm_sb = [ptile([P, V], mm_dt, name=f"wlm{d}") for d in range(nD)]
        ident_sb = ptile([P, P], mm_dt, name="ident")
        maskt_sb = ptile([P, P], mm_dt, name="maskt")
        x_f32 = [ptile([P, D], F32, name=f"xf{t}") for t in range(nTB)]
        # x_bf / xT double-buffered by layer parity (p4 writes the other one)
        x_bf = [[ptile([P, D], mm_dt, name=f"xb{pp}_{t}") for t in range(nTB)]
                for pp in range(2)]
        xT_bf = [[ptile([P, T], mm_dt, name=f"xT{pp}_{d}") for d in range(nD)]
                 for pp in range(2)]
        qrT = [ptile([P, T], mm_dt, name=f"qrT{k}") for k in range(nK)]
        ykvT = [ptile([P, T], mm_dt, name=f"ykvT{d}") for d in range(nD)]
        eps_sb = ptile([P, 1], F32, name="epsb")
        nc.vector.memset(eps_sb[:], EPS)

        # per-layer DRAM bounce buffers for the chunked AllReduce (bf16)
        p_loc = [ptile([T, D], mm_dt, space="DRAM", name=f"ploc{l}")
                 for l in range(NL)]
        p_sum = [ptile([T, D], mm_dt, space="DRAM", addr_space="Shared",
                       name=f"psum{l}") for l in range(NL)]

        # transient pools
        _cms = [tc.tile_pool(name="sp2", bufs=4),
                tc.tile_pool(name="spB", bufs=3),
                tc.tile_pool(name="spX", bufs=16),
                tc.tile_pool(name="spT", bufs=2),
                tc.tile_pool(name="spP", bufs=2),
                tc.tile_pool(name="ppb", bufs=4, space="PSUM"),
                tc.tile_pool(name="pps", bufs=4, space="PSUM")]
        sp2, spB, spX, spT, spP, ppb, pps = [cm.__enter__() for cm in _cms]

        def ln_chain(src_ap, outs):
            """LN over free dim D; scalar engine does the [P,D] elementwise
            work (Square+accum, then Identity with [P,1] scale/bias)."""
            s1 = sp2.tile([P, 1], F32, tag="ln1", name="s1")
            nc.vector.reduce_sum(s1[:], src_ap, axis=mybir.AxisListType.X)
            nm = sp2.tile([P, 1], F32, tag="ln2", name="nm")
            nc.vector.tensor_scalar_mul(nm[:], s1[:], -1.0 / D)
            sq = sp2.tile([P, D], BF16, tag="lnsq", name="sq", bufs=2)
            ss = sp2.tile([P, 1], F32, tag="ln3", name="ss")
            nc.scalar.activation(sq[:], src_ap, AF.Square, bias=nm[:],
                                 accum_out=ss[:])
            sd = sp2.tile([P, 1], F32, tag="ln4", name="sd")
            nc.scalar.activation(sd[:], ss[:], AF.Sqrt, bias=eps_sb[:],
                                 scale=1.0 / D)
            rs = sp2.tile([P, 1], F32, tag="ln5", name="rs")
            nc.vector.reciprocal(rs[:], sd[:])
            rsnm = sp2.tile([P, 1], F32, tag="ln6", name="rsnm")
            nc.vector.tensor_scalar_mul(rsnm[:], rs[:], nm[:])
            for o in outs:
                nc.scalar.activation(o, src_ap, AF.Identity, bias=rsnm[:],
                                     scale=rs[:])

        # ---- setup: load weights, x0; build x_bf and xT (parity 0) ----
        with nc.named_scope("setup"):
            for d in range(nD):
                nc.sync.dma_start(wenc_sb[d][:], wenc_d[d * P:(d + 1) * P, :])
                nc.sync.dma_start(wencv_sb[d][:], wencv_d[d * P:(d + 1) * P, :])
                nc.sync.dma_start(wlm_sb[d][:], wlm_d[d * P:(d + 1) * P, :])
            for k in range(nK):
                nc.sync.dma_start(wdec_sb[k][:], wdec_d[k * P:(k + 1) * P, :])
            nc.sync.dma_start(ident_sb[:], ident_d[:, :])
            nc.sync.dma_start(maskt_sb[:], maskt_d[:, :])
            for t in range(nTB):
                nc.sync.dma_start(x_f32[t][:], x0_d[t * P:(t + 1) * P, :])
                nc.vector.tensor_copy(x_bf[0][t][:], x_f32[t][:])
                for d in range(nD):
                    trp = pps.tile([P, P], mm_dt, tag="acc", name="trp")
                    nc.tensor.transpose(trp[:],
                                        x_bf[0][t][:, d * P:(d + 1) * P],
                                        ident_sb[:])
                    nc.vector.tensor_copy(xT_bf[0][d][:, t * P:(t + 1) * P],
                                          trp[:])

        def emit_p1_unit(l, ts, j):
            """relu(enc^T x) + rope for pair-tile j of chunk (l, ts)."""
            xTl = xT_bf[l % 2]
            c0, c1 = ts * SUP, (ts + 1) * SUP
            ct = spT.tile([P, SUP], mm_dt, tag="ctc", name="ct")
            st = spT.tile([P, SUP], mm_dt, tag="stc", name="st")
            nc.sync.dma_start(ct[:], ctab_d[j * P:(j + 1) * P, c0:c1])
            nc.sync.dma_start(st[:], stab_d[j * P:(j + 1) * P, c0:c1])
            psA = ppb.tile([P, SUP], F32, tag="big", name="psA")
            psB = ppb.tile([P, SUP], F32, tag="big", name="psB")
            for d in range(nD):
                nc.tensor.matmul(psA[:], wenc_sb[d][:, j * P:(j + 1) * P],
                                 xTl[d][:, c0:c1],
                                 start=(d == 0), stop=(d == nD - 1))
            for d in range(nD):
                nc.tensor.matmul(psB[:],
                                 wenc_sb[d][:, (j + nJ) * P:(j + nJ + 1) * P],
                                 xTl[d][:, c0:c1],
                                 start=(d == 0), stop=(d == nD - 1))
            xsA = spB.tile([P, SUP], mm_dt, tag="xsA", name="xsA", bufs=2)
            xsB = spB.tile([P, SUP], mm_dt, tag="xsB", name="xsB", bufs=2)
            nc.scalar.activation(xsA[:], psA[:], AF.Relu)
            nc.scalar.activation(xsB[:], psB[:], AF.Relu)
            if j < ROPE_DVE_J:
                eng, rtag = nc.vector, "rtv"
            else:
                eng, rtag = nc.gpsimd, "rtg"
            t0 = spB.tile([P, SUP], mm_dt, tag=rtag, name="t0", bufs=2)
            t1 = spB.tile([P, SUP], mm_dt, tag=rtag, name="t1", bufs=2)
            eng.tensor_tensor(t0[:], xsA[:], ct[:], ALU.mult)
            eng.tensor_tensor(t1[:], xsB[:], st[:], ALU.mult)
            eng.tensor_tensor(qrT[j][:, c0:c1], t0[:], t1[:], ALU.subtract)
            t2 = spB.tile([P, SUP], mm_dt, tag=rtag, name="t2", bufs=2)
            t3 = spB.tile([P, SUP], mm_dt, tag=rtag, name="t3", bufs=2)
            eng.tensor_tensor(t2[:], xsB[:], ct[:], ALU.mult)
            eng.tensor_tensor(t3[:], xsA[:], st[:], ALU.mult)
            eng.tensor_tensor(qrT[j + nJ][:, c0:c1], t2[:], t3[:], ALU.add)

        def emit_p4(l, ts):
            nxt = (l + 1) % 2
            xbn, xTn = x_bf[nxt], xT_bf[nxt]
            c0, c1 = ts * SUP, (ts + 1) * SUP
            with nc.named_scope(f"l{l}c{ts}_p4"):
                pin = spP.tile([P, nQ * D], mm_dt, tag="pin", name="pin")
                nc.sync.dma_start(
                    pin[:].rearrange("p (n d) -> p n d", n=nQ),
                    p_sum[l][c0:c1, :].rearrange("(n p) d -> p n d", p=P))
                for q in range(nQ):
                    t = nQ * ts + q
                    xr = sp2.tile([P, D], F32, tag="lnr", name="xr")
                    nc.vector.tensor_tensor(xr[:], x_f32[t][:],
                                            pin[:, q * D:(q + 1) * D],
                                            ALU.add)
                    ln_chain(xr[:], [x_f32[t][:], xbn[t][:]])
                    for d in range(nD):
                        trp = pps.tile([P, P], mm_dt, tag="acc", name="trp4")
                        nc.tensor.transpose(
                            trp[:], xbn[t][:, d * P:(d + 1) * P],
                            ident_sb[:])
                        nc.vector.tensor_copy(
                            xTn[d][:, t * P:(t + 1) * P], trp[:])

        def emit_iteration(g):
            l, ts = divmod(g, nTS)
            par = l % 2
            xb = x_bf[par]
            xTl = xT_bf[par]
            c0, c1 = ts * SUP, (ts + 1) * SUP
            nsb = nQ * ts + nQ

            # prefetch target: chunk g+1 (maybe next layer).  Cross-layer
            # rope overwrites qrT cols this chunk's first nQ score units
            # still read as lhsT, so those units must be emitted first.
            if g + 1 < nG:
                l2, ts2 = divmod(g + 1, nTS)
                if l2 != l and not XLAYER_PREFETCH:
                    l2 = None
            else:
                l2 = None
            p1_off = nQ if (l2 is not None and l2 != l) else 0

            with nc.named_scope(f"l{l}c{ts}_p12"):
                if g == 0 or (ts == 0 and not XLAYER_PREFETCH):
                    for j in range(nJ):
                        emit_p1_unit(l, ts, j)

                ykv_ps = [pps.tile([P, D], F32, tag="acc", name=f"ykv{q}")
                          for q in range(nQ)]
                pend = [None]

                def flush_ykv():
                    if pend[0] is None:
                        return
                    sb, stsb, q0 = pend[0]
                    for q in range(q0, nQ):
                        nc.tensor.matmul(
                            ykv_ps[q][:], stsb[:, q * P:(q + 1) * P],
                            xb[sb][:],
                            start=(sb == 0), stop=(sb == nQ * ts + q))
                    pend[0] = None

                for u in range(max(nsb, p1_off + nJ)):
                    if u < nsb:
                        sb = u
                        r = sb - nQ * ts
                        q0 = max(0, r)
                        st_ps = ppb.tile([P, SUP], F32, tag="big",
                                         name="st_ps")
                        dst = st_ps[:, q0 * P:SUP]
                        for k in range(nK):
                            nc.tensor.matmul(
                                dst, qrT[k][:, sb * P:(sb + 1) * P],
                                qrT[k][:, c0 + q0 * P:c0 + SUP],
                                start=(k == 0), stop=(k == nK - 1))
                        flush_ykv()
                        st_sb = spB.tile([P, SUP], mm_dt, tag="stsb",
                                         name="st_sb")
                        if r >= 0:
                            nc.vector.tensor_tensor(
                                st_sb[:, r * P:(r + 1) * P],
                                st_ps[:, r * P:(r + 1) * P], maskt_sb[:],
                                ALU.mult)
                            if r + 1 < nQ:
                                nc.scalar.copy(st_sb[:, (r + 1) * P:SUP],
                                               st_ps[:, (r + 1) * P:SUP])
                        else:
                            nc.scalar.copy(st_sb[:], st_ps[:])
                        pend[0] = (sb, st_sb, q0)
                    if l2 is not None and p1_off <= u < p1_off + nJ:
                        emit_p1_unit(l2, ts2, u - p1_off)
                flush_ykv()

            # ---- ykv LN (no PE), then p3 x_sparse recompute, then the
            # ykv transposes, then y_sparse / xy / p ----
            with nc.named_scope(f"l{l}c{ts}_p3"):
                ykv_n = [sp2.tile([P, D], mm_dt, tag="ykvn", name=f"ykvn{q}")
                         for q in range(nQ)]
                for q in range(nQ):
                    ln_chain(ykv_ps[q][:], [ykv_n[q][:]])

                xsr = []
                for k in range(nK):
                    xs_ps = ppb.tile([P, SUP], F32, tag="big", name="xs_ps")
                    for d in range(nD):
                        nc.tensor.matmul(
                            xs_ps[:], wenc_sb[d][:, k * P:(k + 1) * P],
                            xTl[d][:, c0:c1],
                            start=(d == 0), stop=(d == nD - 1))
                    xt = spX.tile([P, SUP], mm_dt, tag="xsr", name="xsr")
                    if k % 2 == 0:
                        nc.vector.tensor_relu(xt[:], xs_ps[:])
                    else:
                        nc.scalar.activation(xt[:], xs_ps[:], AF.Relu)
                    xsr.append(xt)

                for q in range(nQ):
                    tb = nQ * ts + q
                    for d in range(nD):
                        trp = pps.tile([P, P], mm_dt, tag="acc", name="trp2")
                        nc.tensor.transpose(trp[:],
                                            ykv_n[q][:, d * P:(d + 1) * P],
                                            ident_sb[:])
                        nc.vector.tensor_copy(
                            ykvT[d][:, tb * P:(tb + 1) * P], trp[:])

                YLAG = 3
                p_ps = [pps.tile([P, D], F32, tag="acc", name=f"pp{q}")
                        for q in range(nQ)]
                ys_ps = [None] * nK

                def emit_ys(k):
                    ys_ps[k] = ppb.tile([P, SUP], F32, tag="big", name="ys_ps")
                    for d in range(nD):
                        nc.tensor.matmul(
                            ys_ps[k][:], wencv_sb[d][:, k * P:(k + 1) * P],
                            ykvT[d][:, c0:c1],
                            start=(d == 0), stop=(d == nD - 1))

                for k in range(min(YLAG, nK)):
                    emit_ys(k)
                for k in range(nK):
                    ysr = spB.tile([P, SUP], mm_dt, tag="ysr", name="ysr")
                    nc.scalar.activation(ysr[:], ys_ps[k][:], AF.Relu)
                    xy = spB.tile([P, SUP], mm_dt, tag="xy", name="xy")
                    nc.vector.tensor_tensor(xy[:], ysr[:], xsr[k][:],
                                            ALU.mult)
                    for q in range(nQ):
                        nc.tensor.matmul(
                            p_ps[q][:], xy[:, q * P:(q + 1) * P],
                            wdec_sb[k][:],
                            start=(k == 0), stop=(k == nK - 1))
                    if k + YLAG < nK:
                        emit_ys(k + YLAG)
                pch = spP.tile([P, nQ * D], mm_dt, tag="pch", name="pch")
                for q in range(nQ):
                    nc.scalar.copy(pch[:, q * D:(q + 1) * D], p_ps[q][:])

            # ---- AR: ship p rows, AllReduce this chunk ----
            with nc.named_scope(f"l{l}c{ts}_ar"):
                nc.sync.dma_start(
                    p_loc[l][c0:c1, :].rearrange("(n p) d -> p n d", p=P),
                    pch[:].rearrange("p (n d) -> p n d", n=nQ))
                nc.gpsimd.collective_compute(
                    "AllReduce", ALU.add, replica_groups=RG,
                    ins=[p_loc[l][c0:c1, :]], outs=[p_sum[l][c0:c1, :]])

            if g >= DEFER:
                emit_p4(*divmod(g - DEFER, nTS))

        for g in range(nG):
            emit_iteration(g)
        for g in range(nG - DEFER, nG):
            emit_p4(*divmod(g, nTS))

        # ---- final: out = x @ lm_head (chunk-staged, 4 DMAs) ----
        with nc.named_scope("final"):
            xTf = xT_bf[NL % 2]
            for ts in range(nTS):
                och = spP.tile([P, nQ * V], F32, tag="och", name="och", bufs=1)
                for q in range(nQ):
                    t = nQ * ts + q
                    o_ps = pps.tile([P, V], F32, tag="acc", name="o_ps")
                    for d in range(nD):
                        nc.tensor.matmul(o_ps[:],
                                         xTf[d][:, t * P:(t + 1) * P],
                                         wlm_sb[d][:],
                                         start=(d == 0), stop=(d == nD - 1))
                    nc.vector.tensor_copy(och[:, q * V:(q + 1) * V], o_ps[:])
                nc.sync.dma_start(
                    out_d[ts * SUP:(ts + 1) * SUP, :].rearrange(
                        "(n p) v -> p n v", p=P),
                    och[:].rearrange("p (n v) -> p n v", n=nQ))

        for cm in reversed(_cms):
            cm.__exit__(None, None, None)
        for f in reversed(_keep):
            f()
        _keep.clear()

    nc.compile()
    return nc


def host_inputs(idx, embed, encoder, encoder_v, decoder, lm_head, cfg,
                mm_dt=BF16):
    """Build the 8 per-core input maps (host-side prep is O(MB) copies)."""
    T, D, N, NL, V = cfg["T"], cfg["D"], cfg["N"], cfg["NL"], cfg["V"]
    NH, B = cfg["NH"], cfg["B"]
    np_mm = np.dtype(mybir.dt.np(mm_dt))

    idx = np.asarray(idx)
    embed = np.asarray(embed, dtype=np.float32)
    encoder = np.asarray(encoder, dtype=np.float32)
    encoder_v = np.asarray(encoder_v, dtype=np.float32)
    decoder = np.asarray(decoder, dtype=np.float32)
    lm_head = np.asarray(lm_head, dtype=np.float32)

    # initial x = ln(embed[idx]) in f32 (cheap: B*T*D)
    e = embed[idx]  # (B, T, D)
    mu = e.mean(-1, keepdims=True)
    var = ((e - mu) ** 2).mean(-1, keepdims=True)
    x0 = ((e - mu) / np.sqrt(var + EPS)).astype(np.float32)

    # rope tables in pair-permuted transposed layout [N/2, T]
    theta = np.float32(2.0 ** 16)
    q = (np.floor(np.arange(N, dtype=np.float32) / 2.0) * 2.0).astype(np.float32)
    freqs = (1.0 / (theta ** (q / np.float32(N))) /
             np.float32(2.0 * math.pi)).astype(np.float32)
    fp = freqs[0::2]  # (N/2,)
    ph = fp[:, None] * np.arange(T, dtype=np.float32)[None, :]
    pm = ((ph % np.float32(1.0)) * np.float32(2.0 * math.pi)).astype(np.float32)
    ctab = np.cos(pm).astype(np_mm)
    stab = np.sin(pm).astype(np_mm)

    perm = np.concatenate([np.arange(0, N, 2), np.arange(1, N, 2)])
    ident = np.eye(P, dtype=np_mm)
    maskt = np.triu(np.ones((P, P), np.float32), k=1).astype(np_mm)  # s < t

    in_maps = []
    for c in range(B * NH):
        b, h = c // NH, c % NH
        in_maps.append({
            "x0": x0[b],
            "wenc": encoder[h][:, perm].astype(np_mm),
            "wencv": encoder_v[h][:, perm].astype(np_mm),
            "wdec": decoder[h * N:(h + 1) * N, :][perm, :].astype(np_mm),
            "wlm": lm_head.astype(np_mm),
            "ctab": ctab,
            "stab": stab,
            "ident": ident,
            "maskt": maskt,
        })
    return in_maps


_NC_CACHE = {}


def _get_nc(cfg_key, cfg, mm_dt, n_cores):
    if cfg_key not in _NC_CACHE:
        _NC_CACHE[cfg_key] = build_nc(cfg, mm_dt=mm_dt, n_cores=n_cores)
    return _NC_CACHE[cfg_key]


def kernel(idx, embed, encoder, encoder_v, decoder, lm_head):
    cfg = FULL_CFG
    NH, B = cfg["NH"], cfg["B"]
    n_cores = B * NH
    in_maps = host_inputs(idx, embed, encoder, encoder_v, decoder, lm_head, cfg)
    nc = _get_nc("full_bf16", cfg, BF16, n_cores)
    res = run_bass_kernel_spmd(nc, in_maps, core_ids=list(range(n_cores)))
    out = np.stack([np.asarray(res.results[b * NH]["out"], dtype=np.float32)
                    for b in range(B)], axis=0)
    return out


# revision 14
# speedup vs baseline: 1.0120x; 1.0120x over previous
"""Trainium2 Bass kernel for the sparse-attention nn module (nn_BDH_48421461295735).

Strategy: 8 NeuronCores = 8 (batch, head) pairs (B=2 x NH=4).  Each core runs
all 4 layers for its head; the only cross-core traffic is a per-layer
AllReduce (within each batch's group of 4 cores) of the per-head decoder
partial p = (x_sparse*y_sparse) @ dec_h, chunked by 512 token rows (bf16) so
it pipelines behind compute.

The work is one global software pipeline over chunk iterations
g = (layer, ts) of 512 token columns:
  - score units (chunk g) interleaved with p1 units (chunk g+1):
    p1 unit j: x_sparseT = relu(enc^T xT) (scalar relu), rope via 6
    tensor_tensor ops (DVE for j<5, gpsimd for j>=5) -> qrT cols.
    score unit sb: causal score tile qrT^T qrT -> PSUM, strict mask on the
    diagonal (DVE), PSUM->SBUF copy on the scalar engine, ykv accumulation
    lagged one unit.
  - ykv LN (scalar-engine heavy: Square+accum, Sqrt, Identity with [P,1]
    scale/bias), then p3 x_sparse recompute (relus alternating DVE/scalar,
    results held in a 16-deep pool), ykv transposes, then y_sparse / xy / p
    with ys-matmuls emitted 3 k-steps ahead of the p matmuls.
  - AR: DMA p rows -> DRAM bf16, 4-core AllReduce chunk.
  - p4 for chunk g-2 (defer gives the AllReduce ~2 chunks of compute to
    hide): x' = ln(x + p_sum) -> x ping-pong buffers + xT transposes.
x_bf / xT are double-buffered by layer parity so p4 can run while later
chunks of the same layer still read the old x.

The neuron axis is host-permuted (evens then odds) so the rope pair partner
of partition-tile j is partition-tile j+nJ (no cross-partition shuffles).
"""

import math
import sys

import numpy as np

for _p in ("/opt/trn_rl_repo",):
    if _p not in sys.path:
        sys.path.insert(0, _p)

import concourse.bass as bass
import concourse.bacc as bacc
import concourse.mybir as mybir
import concourse.tile as tile
from concourse.bass_utils import run_bass_kernel_spmd

F32 = mybir.dt.float32
BF16 = mybir.dt.bfloat16
AF = mybir.ActivationFunctionType
ALU = mybir.AluOpType

FULL_CFG = dict(T=2048, D=256, N=2048, NL=4, V=256, NH=4, B=2)
P = 128
SUP = 512
EPS = 1e-5


def build_nc(cfg, mm_dt=BF16, n_cores=8):
    T, D, N, NL, V = cfg["T"], cfg["D"], cfg["N"], cfg["NL"], cfg["V"]
    NH = cfg["NH"]
    assert T % SUP == 0 and D % P == 0 and N % 256 == 0 and V == D
    nTB, nTS, nD, nK = T // P, T // SUP, D // P, N // P
    nJ = nK // 2
    nQ = SUP // P  # 4
    nG = NL * nTS
    # p4 deferral in chunk iterations; the cross-layer p1 prefetch of chunk
    # (l+1, 0) at iteration (l, nTS-1) reads xT written by p4(l, 0), which
    # is emitted at the end of iteration (l, DEFER) -- so DEFER <= nTS-2.
    XLAYER_PREFETCH = nTS >= 2
    DEFER = max(0, min(2, nTS - 2)) if XLAYER_PREFETCH else 0
    ROPE_DVE_J = max(1, (nJ * 5 + 7) // 8)  # j < this -> DVE, else gpsimd

    nc = bacc.Bacc("TRN2", target_bir_lowering=False, debug=False,
                   num_devices=n_cores)

    x0_d = nc.dram_tensor("x0", [T, D], F32, kind="ExternalInput")
    x0b_d = nc.dram_tensor("x0b", [T, D], mm_dt, kind="ExternalInput")
    x0T_d = nc.dram_tensor("x0T", [D, T], mm_dt, kind="ExternalInput")
    wenc_d = nc.dram_tensor("wenc", [D, N], mm_dt, kind="ExternalInput")
    wencv_d = nc.dram_tensor("wencv", [D, N], mm_dt, kind="ExternalInput")
    wdec_d = nc.dram_tensor("wdec", [N, D], mm_dt, kind="ExternalInput")
    wlm_d = nc.dram_tensor("wlm", [D, V], mm_dt, kind="ExternalInput")
    ctab_d = nc.dram_tensor("ctab", [N // 2, T], mm_dt, kind="ExternalInput")
    stab_d = nc.dram_tensor("stab", [N // 2, T], mm_dt, kind="ExternalInput")
    ident_d = nc.dram_tensor("ident", [P, P], mm_dt, kind="ExternalInput")
    maskt_d = nc.dram_tensor("maskt", [P, P], mm_dt, kind="ExternalInput")
    out_d = nc.dram_tensor("out", [T, V], F32, kind="ExternalOutput")

    # AllReduce groups: one group of NH cores per batch.
    RG = [list(range(g * NH, (g + 1) * NH)) for g in range(n_cores // NH)]

    with tile.TileContext(nc) as tc:
        _keep = []  # keep tc.tile free-closures alive (GC would release pools)

        def ptile(shape, dtype, name, **kw):
            t, free = tc.tile(shape, dtype, name=name, **kw)
            _keep.append(free)
            return t

        # ---- persistent SBUF tensors ----
        wenc_sb = [ptile([P, N], mm_dt, name=f"wenc{d}") for d in range(nD)]
        wencv_sb = [ptile([P, N], mm_dt, name=f"wencv{d}") for d in range(nD)]
        wdec_sb = [ptile([P, D], mm_dt, name=f"wdec{k}") for k in range(nK)]
        wlm_sb = [ptile([P, V], mm_dt, name=f"wlm{d}") for d in range(nD)]
        ident_sb = ptile([P, P], mm_dt, name="ident")
        maskt_sb = ptile([P, P], mm_dt, name="maskt")
        x_f32 = [ptile([P, D], F32, name=f"xf{t}") for t in range(nTB)]
        # x_bf / xT double-buffered by layer parity (p4 writes the other one)
        x_bf = [[ptile([P, D], mm_dt, name=f"xb{pp}_{t}") for t in range(nTB)]
                for pp in range(2)]
        xT_bf = [[ptile([P, T], mm_dt, name=f"xT{pp}_{d}") for d in range(nD)]
                 for pp in range(2)]
        qrT = [ptile([P, T], mm_dt, name=f"qrT{k}") for k in range(nK)]
        ykvT = [ptile([P, T], mm_dt, name=f"ykvT{d}") for d in range(nD)]
        eps_sb = ptile([P, 1], F32, name="epsb")
        nc.vector.memset(eps_sb[:], EPS)

        # per-layer DRAM bounce buffers for the chunked AllReduce (bf16)
        p_loc = [ptile([T, D], mm_dt, space="DRAM", name=f"ploc{l}")
                 for l in range(NL)]
        p_sum = [ptile([T, D], mm_dt, space="DRAM", addr_space="Shared",
                       name=f"psum{l}") for l in range(NL)]
        # warmup-collective buffers (see setup)
        wu_sb = ptile([1, 256], mm_dt, name="wu")
        wu_loc = ptile([1, 256], mm_dt, space="DRAM", name="wuloc")
        wu_sum = ptile([1, 256], mm_dt, space="DRAM", addr_space="Shared",
                       name="wusum")

        # transient pools
        _cms = [tc.tile_pool(name="sp2", bufs=4),
                tc.tile_pool(name="spB", bufs=3),
                tc.tile_pool(name="spX", bufs=16),
                tc.tile_pool(name="spT", bufs=2),
                tc.tile_pool(name="spP", bufs=2),
                tc.tile_pool(name="ppb", bufs=4, space="PSUM"),
                tc.tile_pool(name="pps", bufs=4, space="PSUM")]
        sp2, spB, spX, spT, spP, ppb, pps = [cm.__enter__() for cm in _cms]

        def ln_chain(src_ap, outs):
            """LN over free dim D; scalar engine does the [P,D] elementwise
            work (Square+accum, then Identity with [P,1] scale/bias)."""
            s1 = sp2.tile([P, 1], F32, tag="ln1", name="s1")
            nc.vector.reduce_sum(s1[:], src_ap, axis=mybir.AxisListType.X)
            nm = sp2.tile([P, 1], F32, tag="ln2", name="nm")
            nc.vector.tensor_scalar_mul(nm[:], s1[:], -1.0 / D)
            sq = sp2.tile([P, D], BF16, tag="lnsq", name="sq", bufs=2)
            ss = sp2.tile([P, 1], F32, tag="ln3", name="ss")
            nc.scalar.activation(sq[:], src_ap, AF.Square, bias=nm[:],
                                 accum_out=ss[:])
            sd = sp2.tile([P, 1], F32, tag="ln4", name="sd")
            nc.scalar.activation(sd[:], ss[:], AF.Sqrt, bias=eps_sb[:],
                                 scale=1.0 / D)
            rs = sp2.tile([P, 1], F32, tag="ln5", name="rs")
            nc.vector.reciprocal(rs[:], sd[:])
            rsnm = sp2.tile([P, 1], F32, tag="ln6", name="rsnm")
            nc.vector.tensor_scalar_mul(rsnm[:], rs[:], nm[:])
            for o in outs:
                nc.scalar.activation(o, src_ap, AF.Identity, bias=rsnm[:],
                                     scale=rs[:])

        # ---- setup: load weights, x0 (+ host-transposed copies).  The
        # warmup collective initializes the CC channel during setup so the
        # first real AllReduce runs at steady-state latency. ----
        with nc.named_scope("setup"):
            nc.vector.memset(wu_sb[:], 0.0)
            nc.sync.dma_start(wu_loc[:, :], wu_sb[:])
            nc.gpsimd.collective_compute(
                "AllReduce", ALU.add, replica_groups=RG,
                ins=[wu_loc[:, :]], outs=[wu_sum[:, :]])
            for d in range(nD):
                nc.sync.dma_start(xT_bf[0][d][:],
                                  x0T_d[d * P:(d + 1) * P, :])
                nc.sync.dma_start(wenc_sb[d][:], wenc_d[d * P:(d + 1) * P, :])
                nc.sync.dma_start(wencv_sb[d][:], wencv_d[d * P:(d + 1) * P, :])
                nc.sync.dma_start(wlm_sb[d][:], wlm_d[d * P:(d + 1) * P, :])
            for k in range(nK):
                nc.sync.dma_start(wdec_sb[k][:], wdec_d[k * P:(k + 1) * P, :])
            nc.sync.dma_start(ident_sb[:], ident_d[:, :])
            nc.sync.dma_start(maskt_sb[:], maskt_d[:, :])
            for t in range(nTB):
                nc.sync.dma_start(x_bf[0][t][:], x0b_d[t * P:(t + 1) * P, :])
                nc.sync.dma_start(x_f32[t][:], x0_d[t * P:(t + 1) * P, :])

        def emit_p1_unit(l, ts, j):
            """relu(enc^T x) + rope for pair-tile j of chunk (l, ts)."""
            xTl = xT_bf[l % 2]
            c0, c1 = ts * SUP, (ts + 1) * SUP
            ct = spT.tile([P, SUP], mm_dt, tag="ctc", name="ct")
            st = spT.tile([P, SUP], mm_dt, tag="stc", name="st")
            nc.sync.dma_start(ct[:], ctab_d[j * P:(j + 1) * P, c0:c1])
            nc.sync.dma_start(st[:], stab_d[j * P:(j + 1) * P, c0:c1])
            psA = ppb.tile([P, SUP], F32, tag="big", name="psA")
            psB = ppb.tile([P, SUP], F32, tag="big", name="psB")
            for d in range(nD):
                nc.tensor.matmul(psA[:], wenc_sb[d][:, j * P:(j + 1) * P],
                                 xTl[d][:, c0:c1],
                                 start=(d == 0), stop=(d == nD - 1))
            for d in range(nD):
                nc.tensor.matmul(psB[:],
                                 wenc_sb[d][:, (j + nJ) * P:(j + nJ + 1) * P],
                                 xTl[d][:, c0:c1],
                                 start=(d == 0), stop=(d == nD - 1))
            xsA = spB.tile([P, SUP], mm_dt, tag="xsA", name="xsA", bufs=2)
            xsB = spB.tile([P, SUP], mm_dt, tag="xsB", name="xsB", bufs=2)
            nc.scalar.activation(xsA[:], psA[:], AF.Relu)
            nc.scalar.activation(xsB[:], psB[:], AF.Relu)
            if j < ROPE_DVE_J:
                eng, rtag = nc.vector, "rtv"
            else:
                eng, rtag = nc.gpsimd, "rtg"
            t0 = spB.tile([P, SUP], mm_dt, tag=rtag, name="t0", bufs=2)
            t1 = spB.tile([P, SUP], mm_dt, tag=rtag, name="t1", bufs=2)
            eng.tensor_tensor(t0[:], xsA[:], ct[:], ALU.mult)
            eng.tensor_tensor(t1[:], xsB[:], st[:], ALU.mult)
            eng.tensor_tensor(qrT[j][:, c0:c1], t0[:], t1[:], ALU.subtract)
            t2 = spB.tile([P, SUP], mm_dt, tag=rtag, name="t2", bufs=2)
            t3 = spB.tile([P, SUP], mm_dt, tag=rtag, name="t3", bufs=2)
            eng.tensor_tensor(t2[:], xsB[:], ct[:], ALU.mult)
            eng.tensor_tensor(t3[:], xsA[:], st[:], ALU.mult)
            eng.tensor_tensor(qrT[j + nJ][:, c0:c1], t2[:], t3[:], ALU.add)

        def emit_p4(l, ts):
            nxt = (l + 1) % 2
            xbn, xTn = x_bf[nxt], xT_bf[nxt]
            c0, c1 = ts * SUP, (ts + 1) * SUP
            with nc.named_scope(f"l{l}c{ts}_p4"):
                pin = spP.tile([P, nQ * D], mm_dt, tag="pin", name="pin")
                nc.sync.dma_start(
                    pin[:].rearrange("p (n d) -> p n d", n=nQ),
                    p_sum[l][c0:c1, :].rearrange("(n p) d -> p n d", p=P))
                for q in range(nQ):
                    t = nQ * ts + q
                    xr = sp2.tile([P, D], F32, tag="lnr", name="xr")
                    nc.vector.tensor_tensor(xr[:], x_f32[t][:],
                                            pin[:, q * D:(q + 1) * D],
                                            ALU.add)
                    ln_chain(xr[:], [x_f32[t][:], xbn[t][:]])
                    for d in range(nD):
                        trp = pps.tile([P, P], mm_dt, tag="acc", name="trp4")
                        nc.tensor.transpose(
                            trp[:], xbn[t][:, d * P:(d + 1) * P],
                            ident_sb[:])
                        nc.vector.tensor_copy(
                            xTn[d][:, t * P:(t + 1) * P], trp[:])

        def emit_iteration(g):
            l, ts = divmod(g, nTS)
            par = l % 2
            xb = x_bf[par]
            xTl = xT_bf[par]
            c0, c1 = ts * SUP, (ts + 1) * SUP
            nsb = nQ * ts + nQ

            # prefetch target: chunk g+1 (maybe next layer).  Cross-layer
            # rope overwrites qrT cols this chunk's first nQ score units
            # still read as lhsT, so those units must be emitted first.
            if g + 1 < nG:
                l2, ts2 = divmod(g + 1, nTS)
                if l2 != l and not XLAYER_PREFETCH:
                    l2 = None
            else:
                l2 = None
            p1_off = nQ if (l2 is not None and l2 != l) else 0

            with nc.named_scope(f"l{l}c{ts}_p12"):
                if g == 0 or (ts == 0 and not XLAYER_PREFETCH):
                    for j in range(nJ):
                        emit_p1_unit(l, ts, j)

                ykv_ps = [pps.tile([P, D], F32, tag="acc", name=f"ykv{q}")
                          for q in range(nQ)]
                pend = [None]

                def flush_ykv():
                    if pend[0] is None:
                        return
                    sb, stsb, q0 = pend[0]
                    for q in range(q0, nQ):
                        nc.tensor.matmul(
                            ykv_ps[q][:], stsb[:, q * P:(q + 1) * P],
                            xb[sb][:],
                            start=(sb == 0), stop=(sb == nQ * ts + q))
                    pend[0] = None

                for u in range(max(nsb, p1_off + nJ)):
                    if u < nsb:
                        sb = u
                        r = sb - nQ * ts
                        q0 = max(0, r)
                        st_ps = ppb.tile([P, SUP], F32, tag="big",
                                         name="st_ps")
                        dst = st_ps[:, q0 * P:SUP]
                        for k in range(nK):
                            nc.tensor.matmul(
                                dst, qrT[k][:, sb * P:(sb + 1) * P],
                                qrT[k][:, c0 + q0 * P:c0 + SUP],
                                start=(k == 0), stop=(k == nK - 1))
                        flush_ykv()
                        st_sb = spB.tile([P, SUP], mm_dt, tag="stsb",
                                         name="st_sb")
                        if r >= 0:
                            nc.vector.tensor_tensor(
                                st_sb[:, r * P:(r + 1) * P],
                                st_ps[:, r * P:(r + 1) * P], maskt_sb[:],
                                ALU.mult)
                            if r + 1 < nQ:
                                nc.scalar.copy(st_sb[:, (r + 1) * P:SUP],
                                               st_ps[:, (r + 1) * P:SUP])
                        else:
                            nc.scalar.copy(st_sb[:], st_ps[:])
                        pend[0] = (sb, st_sb, q0)
                    if l2 is not None and p1_off <= u < p1_off + nJ:
                        emit_p1_unit(l2, ts2, u - p1_off)
                flush_ykv()

            # ---- ykv LN (no PE), then p3 x_sparse recompute, then the
            # ykv transposes, then y_sparse / xy / p ----
            with nc.named_scope(f"l{l}c{ts}_p3"):
                ykv_n = [sp2.tile([P, D], mm_dt, tag="ykvn", name=f"ykvn{q}")
                         for q in range(nQ)]
                for q in range(nQ):
                    ln_chain(ykv_ps[q][:], [ykv_n[q][:]])

                xsr = []
                for k in range(nK):
                    xs_ps = ppb.tile([P, SUP], F32, tag="big", name="xs_ps")
                    for d in range(nD):
                        nc.tensor.matmul(
                            xs_ps[:], wenc_sb[d][:, k * P:(k + 1) * P],
                            xTl[d][:, c0:c1],
                            start=(d == 0), stop=(d == nD - 1))
                    xt = spX.tile([P, SUP], mm_dt, tag="xsr", name="xsr")
                    if k % 2 == 0:
                        nc.vector.tensor_relu(xt[:], xs_ps[:])
                    else:
                        nc.scalar.activation(xt[:], xs_ps[:], AF.Relu)
                    xsr.append(xt)

                for q in range(nQ):
                    tb = nQ * ts + q
                    for d in range(nD):
                        trp = pps.tile([P, P], mm_dt, tag="acc", name="trp2")
                        nc.tensor.transpose(trp[:],
                                            ykv_n[q][:, d * P:(d + 1) * P],
                                            ident_sb[:])
                        nc.vector.tensor_copy(
                            ykvT[d][:, tb * P:(tb + 1) * P], trp[:])

                YLAG = 3
                p_ps = [pps.tile([P, D], F32, tag="acc", name=f"pp{q}")
                        for q in range(nQ)]
                ys_ps = [None] * nK

                def emit_ys(k):
                    ys_ps[k] = ppb.tile([P, SUP], F32, tag="big", name="ys_ps")
                    for d in range(nD):
                        nc.tensor.matmul(
                            ys_ps[k][:], wencv_sb[d][:, k * P:(k + 1) * P],
                            ykvT[d][:, c0:c1],
                            start=(d == 0), stop=(d == nD - 1))

                for k in range(min(YLAG, nK)):
                    emit_ys(k)
                for k in range(nK):
                    ysr = spB.tile([P, SUP], mm_dt, tag="ysr", name="ysr")
                    nc.scalar.activation(ysr[:], ys_ps[k][:], AF.Relu)
                    xy = spB.tile([P, SUP], mm_dt, tag="xy", name="xy")
                    nc.vector.tensor_tensor(xy[:], ysr[:], xsr[k][:],
                                            ALU.mult)
                    for q in range(nQ):
                        nc.tensor.matmul(
                            p_ps[q][:], xy[:, q * P:(q + 1) * P],
                            wdec_sb[k][:],
                            start=(k == 0), stop=(k == nK - 1))
                    if k + YLAG < nK:
                        emit_ys(k + YLAG)
                pch = spP.tile([P, nQ * D], mm_dt, tag="pch", name="pch")
                for q in range(nQ):
                    nc.scalar.copy(pch[:, q * D:(q + 1) * D], p_ps[q][:])

            # ---- AR: ship p rows, AllReduce this chunk ----
            with nc.named_scope(f"l{l}c{ts}_ar"):
                nc.sync.dma_start(
                    p_loc[l][c0:c1, :].rearrange("(n p) d -> p n d", p=P),
                    pch[:].rearrange("p (n d) -> p n d", n=nQ))
                nc.gpsimd.collective_compute(
                    "AllReduce", ALU.add, replica_groups=RG,
                    ins=[p_loc[l][c0:c1, :]], outs=[p_sum[l][c0:c1, :]])

            if g >= DEFER:
                emit_p4(*divmod(g - DEFER, nTS))

        for g in range(nG):
            emit_iteration(g)
        pending = [divmod(g, nTS) for g in range(nG - DEFER, nG)]

        # ---- final: out = x @ lm_head; pending p4s interleave so the
        # early lm_head chunks cover the last AllReduces ----
        with nc.named_scope("final"):
            if pending:
                emit_p4(*pending[0])
            xTf = xT_bf[NL % 2]
            for ts in range(nTS):
                if ts == nTS - 1:
                    for pc in pending[1:]:
                        emit_p4(*pc)
                och = spP.tile([P, nQ * V], F32, tag="och", name="och", bufs=1)
                for q in range(nQ):
                    t = nQ * ts + q
                    o_ps = pps.tile([P, V], F32, tag="acc", name="o_ps")
                    for d in range(nD):
                        nc.tensor.matmul(o_ps[:],
                                         xTf[d][:, t * P:(t + 1) * P],
                                         wlm_sb[d][:],
                                         start=(d == 0), stop=(d == nD - 1))
                    nc.vector.tensor_copy(och[:, q * V:(q + 1) * V], o_ps[:])
                nc.sync.dma_start(
                    out_d[ts * SUP:(ts + 1) * SUP, :].rearrange(
                        "(n p) v -> p n v", p=P),
                    och[:].rearrange("p (n v) -> p n v", n=nQ))

        for cm in reversed(_cms):
            cm.__exit__(None, None, None)
        for f in reversed(_keep):
            f()
        _keep.clear()

    nc.compile()
    return nc


def host_inputs(idx, embed, encoder, encoder_v, decoder, lm_head, cfg,
                mm_dt=BF16):
    """Build the 8 per-core input maps (host-side prep is O(MB) copies)."""
    T, D, N, NL, V = cfg["T"], cfg["D"], cfg["N"], cfg["NL"], cfg["V"]
    NH, B = cfg["NH"], cfg["B"]
    np_mm = np.dtype(mybir.dt.np(mm_dt))

    idx = np.asarray(idx)
    embed = np.asarray(embed, dtype=np.float32)
    encoder = np.asarray(encoder, dtype=np.float32)
    encoder_v = np.asarray(encoder_v, dtype=np.float32)
    decoder = np.asarray(decoder, dtype=np.float32)
    lm_head = np.asarray(lm_head, dtype=np.float32)

    # initial x = ln(embed[idx]) in f32 (cheap: B*T*D)
    e = embed[idx]  # (B, T, D)
    mu = e.mean(-1, keepdims=True)
    var = ((e - mu) ** 2).mean(-1, keepdims=True)
    x0 = ((e - mu) / np.sqrt(var + EPS)).astype(np.float32)

    # rope tables in pair-permuted transposed layout [N/2, T]
    theta = np.float32(2.0 ** 16)
    q = (np.floor(np.arange(N, dtype=np.float32) / 2.0) * 2.0).astype(np.float32)
    freqs = (1.0 / (theta ** (q / np.float32(N))) /
             np.float32(2.0 * math.pi)).astype(np.float32)
    fp = freqs[0::2]  # (N/2,)
    ph = fp[:, None] * np.arange(T, dtype=np.float32)[None, :]
    pm = ((ph % np.float32(1.0)) * np.float32(2.0 * math.pi)).astype(np.float32)
    ctab = np.cos(pm).astype(np_mm)
    stab = np.sin(pm).astype(np_mm)

    perm = np.concatenate([np.arange(0, N, 2), np.arange(1, N, 2)])
    ident = np.eye(P, dtype=np_mm)
    maskt = np.triu(np.ones((P, P), np.float32), k=1).astype(np_mm)  # s < t

    x0b = x0.astype(np_mm)
    x0T = np.ascontiguousarray(x0b.transpose(0, 2, 1))

    in_maps = []
    for c in range(B * NH):
        b, h = c // NH, c % NH
        in_maps.append({
            "x0": x0[b],
            "x0b": x0b[b],
            "x0T": x0T[b],
            "wenc": encoder[h][:, perm].astype(np_mm),
            "wencv": encoder_v[h][:, perm].astype(np_mm),
            "wdec": decoder[h * N:(h + 1) * N, :][perm, :].astype(np_mm),
            "wlm": lm_head.astype(np_mm),
            "ctab": ctab,
            "stab": stab,
            "ident": ident,
            "maskt": maskt,
        })
    return in_maps


_NC_CACHE = {}


def _get_nc(cfg_key, cfg, mm_dt, n_cores):
    if cfg_key not in _NC_CACHE:
        _NC_CACHE[cfg_key] = build_nc(cfg, mm_dt=mm_dt, n_cores=n_cores)
    return _NC_CACHE[cfg_key]


def kernel(idx, embed, encoder, encoder_v, decoder, lm_head):
    cfg = FULL_CFG
    NH, B = cfg["NH"], cfg["B"]
    n_cores = B * NH
    in_maps = host_inputs(idx, embed, encoder, encoder_v, decoder, lm_head, cfg)
    nc = _get_nc("full_bf16", cfg, BF16, n_cores)
    res = run_bass_kernel_spmd(nc, in_maps, core_ids=list(range(n_cores)))
    out = np.stack([np.asarray(res.results[b * NH]["out"], dtype=np.float32)
                    for b in range(B)], axis=0)
    return out


# revision 24
# speedup vs baseline: 1.0619x; 1.0492x over previous
"""Trainium2 Bass kernel for the sparse-attention nn module (nn_BDH_48421461295735).

Strategy: 8 NeuronCores = 8 (batch, head) pairs (B=2 x NH=4).  Each core runs
all 4 layers for its head; the only cross-core traffic is a per-layer
AllReduce (within each batch's group of 4 cores) of the per-head decoder
partial p = (x_sparse*y_sparse) @ dec_h, chunked by 512 token rows (bf16) so
it pipelines behind compute.

The work is one global software pipeline over chunk iterations
g = (layer, ts) of 512 token columns:
  - score units (chunk g) interleaved with DVE p1 units (chunk g+1):
    p1 unit j: x_sparseT = relu(enc^T xT) into a 2-chunk-deep tile pool
    (reused by p3's xy product -- no recompute), rope via 6 in-place
    tensor_tensor ops (DVE units; the last 3 units run on gpsimd with
    gpsimd relus and are emitted at the start of p3 as PE cover for the
    ykv LN latency).  score unit sb: causal score tile qrT^T qrT -> PSUM,
    strict mask on the diagonal (DVE), PSUM->SBUF copy on the scalar
    engine, ykv accumulation lagged one unit; each ykv row-block is
    LayerNormed as soon as its diagonal block completes (scalar-engine
    Square+accum / Sqrt / Identity with [P,1] scale+bias).
  - p3: ykv transposes, then y_sparse / xy / p with ys-matmuls emitted
    3 k-steps ahead of the p matmuls.
  - AR: DMA p rows -> DRAM f32, 4-core AllReduce chunk (bf16 collectives
    run at half bus bandwidth, so f32 wins); a tiny warmup AllReduce in
    setup absorbs the ~50us CC-channel init.
  - p4 for chunk g-2 (defer gives the AllReduce ~2 chunks of compute to
    hide): x' = ln(x + p_sum) -> x ping-pong buffers + xT transposes.
x_bf / xT are double-buffered by layer parity so p4 can run while later
chunks of the same layer still read the old x.

The neuron axis is host-permuted (evens then odds) so the rope pair partner
of partition-tile j is partition-tile j+nJ (no cross-partition shuffles).
"""

import math
import sys

import numpy as np

for _p in ("/opt/trn_rl_repo",):
    if _p not in sys.path:
        sys.path.insert(0, _p)

import concourse.bass as bass
import concourse.bacc as bacc
import concourse.mybir as mybir
import concourse.tile as tile
from concourse.bass_utils import run_bass_kernel_spmd

F32 = mybir.dt.float32
BF16 = mybir.dt.bfloat16
AF = mybir.ActivationFunctionType
ALU = mybir.AluOpType

FULL_CFG = dict(T=2048, D=256, N=2048, NL=4, V=256, NH=4, B=2)
P = 128
SUP = 512
EPS = 1e-5


def build_nc(cfg, mm_dt=BF16, n_cores=8):
    T, D, N, NL, V = cfg["T"], cfg["D"], cfg["N"], cfg["NL"], cfg["V"]
    NH = cfg["NH"]
    assert T % SUP == 0 and D % P == 0 and N % 256 == 0 and V == D
    nTB, nTS, nD, nK = T // P, T // SUP, D // P, N // P
    nJ = nK // 2
    nQ = SUP // P  # 4
    nG = NL * nTS
    # p4 deferral in chunk iterations; the cross-layer p1 prefetch of chunk
    # (l+1, 0) at iteration (l, nTS-1) reads xT written by p4(l, 0), which
    # is emitted at the end of iteration (l, DEFER) -- so DEFER <= nTS-2.
    XLAYER_PREFETCH = nTS >= 2
    DEFER = max(0, min(2, nTS - 2)) if XLAYER_PREFETCH else 0
    ROPE_DVE_J = max(1, (nJ * 5 + 7) // 8)  # j < this -> DVE, else gpsimd

    nc = bacc.Bacc("TRN2", target_bir_lowering=False, debug=False,
                   num_devices=n_cores)

    x0_d = nc.dram_tensor("x0", [T, D], F32, kind="ExternalInput")
    x0b_d = nc.dram_tensor("x0b", [T, D], mm_dt, kind="ExternalInput")
    x0T_d = nc.dram_tensor("x0T", [D, T], mm_dt, kind="ExternalInput")
    wenc_d = nc.dram_tensor("wenc", [D, N], mm_dt, kind="ExternalInput")
    wencv_d = nc.dram_tensor("wencv", [D, N], mm_dt, kind="ExternalInput")
    wdec_d = nc.dram_tensor("wdec", [N, D], mm_dt, kind="ExternalInput")
    wlm_d = nc.dram_tensor("wlm", [D, V], mm_dt, kind="ExternalInput")
    ctab_d = nc.dram_tensor("ctab", [N // 2, T], mm_dt, kind="ExternalInput")
    stab_d = nc.dram_tensor("stab", [N // 2, T], mm_dt, kind="ExternalInput")
    ident_d = nc.dram_tensor("ident", [P, P], mm_dt, kind="ExternalInput")
    maskt_d = nc.dram_tensor("maskt", [P, P], mm_dt, kind="ExternalInput")
    out_d = nc.dram_tensor("out", [T, V], F32, kind="ExternalOutput")

    # AllReduce groups: one group of NH cores per batch.
    RG = [list(range(g * NH, (g + 1) * NH)) for g in range(n_cores // NH)]

    with tile.TileContext(nc) as tc:
        _keep = []  # keep tc.tile free-closures alive (GC would release pools)

        def ptile(shape, dtype, name, **kw):
            t, free = tc.tile(shape, dtype, name=name, **kw)
            _keep.append(free)
            return t

        # ---- persistent SBUF tensors ----
        wenc_sb = [ptile([P, N], mm_dt, name=f"wenc{d}") for d in range(nD)]
        wencv_sb = [ptile([P, N], mm_dt, name=f"wencv{d}") for d in range(nD)]
        wdec_sb = [ptile([P, D], mm_dt, name=f"wdec{k}") for k in range(nK)]
        wlm_sb = [ptile([P, V], mm_dt, name=f"wlm{d}") for d in range(nD)]
        ident_sb = ptile([P, P], mm_dt, name="ident")
        maskt_sb = ptile([P, P], mm_dt, name="maskt")
        x_f32 = [ptile([P, D], F32, name=f"xf{t}") for t in range(nTB)]
        # x_bf / xT double-buffered by layer parity (p4 writes the other one)
        x_bf = [[ptile([P, D], mm_dt, name=f"xb{pp}_{t}") for t in range(nTB)]
                for pp in range(2)]
        xT_bf = [[ptile([P, T], mm_dt, name=f"xT{pp}_{d}") for d in range(nD)]
                 for pp in range(2)]
        qrT = [ptile([P, T], mm_dt, name=f"qrT{k}") for k in range(nK)]
        ykvT = [ptile([P, T], mm_dt, name=f"ykvT{d}") for d in range(nD)]
        eps_sb = ptile([P, 1], F32, name="epsb")
        nc.vector.memset(eps_sb[:], EPS)

        # per-layer DRAM bounce buffers for the chunked AllReduce.  f32:
        # the CC cores reduce bf16 at half the bus bandwidth, so bf16 is
        # slower end-to-end despite halving the bytes.
        p_loc = [ptile([T, D], F32, space="DRAM", name=f"ploc{l}")
                 for l in range(NL)]
        p_sum = [ptile([T, D], F32, space="DRAM", addr_space="Shared",
                       name=f"psum{l}") for l in range(NL)]
        # warmup-collective buffers (see setup)
        wu_sb = ptile([1, 16], mm_dt, name="wu")
        wu_loc = ptile([1, 16], mm_dt, space="DRAM", name="wuloc")
        wu_sum = ptile([1, 16], mm_dt, space="DRAM", addr_space="Shared",
                       name="wusum")

        # transient pools
        _cms = [tc.tile_pool(name="sp2", bufs=4),
                tc.tile_pool(name="spB", bufs=3),
                tc.tile_pool(name="spX", bufs=2 * nK),
                tc.tile_pool(name="spT", bufs=2),
                tc.tile_pool(name="spP", bufs=2),
                tc.tile_pool(name="ppb", bufs=4, space="PSUM"),
                tc.tile_pool(name="pps", bufs=4, space="PSUM")]
        sp2, spB, spX, spT, spP, ppb, pps = [cm.__enter__() for cm in _cms]

        def ln_chain(src_ap, outs):
            """LN over free dim D; scalar engine does the [P,D] elementwise
            work (Square+accum, then Identity with [P,1] scale/bias)."""
            s1 = sp2.tile([P, 1], F32, tag="ln1", name="s1")
            nc.vector.reduce_sum(s1[:], src_ap, axis=mybir.AxisListType.X)
            nm = sp2.tile([P, 1], F32, tag="ln2", name="nm")
            nc.vector.tensor_scalar_mul(nm[:], s1[:], -1.0 / D)
            sq = sp2.tile([P, D], BF16, tag="lnsq", name="sq", bufs=1)
            ss = sp2.tile([P, 1], F32, tag="ln3", name="ss")
            nc.scalar.activation(sq[:], src_ap, AF.Square, bias=nm[:],
                                 accum_out=ss[:])
            sd = sp2.tile([P, 1], F32, tag="ln4", name="sd")
            nc.scalar.activation(sd[:], ss[:], AF.Sqrt, bias=eps_sb[:],
                                 scale=1.0 / D)
            rs = sp2.tile([P, 1], F32, tag="ln5", name="rs")
            nc.vector.reciprocal(rs[:], sd[:])
            rsnm = sp2.tile([P, 1], F32, tag="ln6", name="rsnm")
            nc.vector.tensor_scalar_mul(rsnm[:], rs[:], nm[:])
            for o in outs:
                nc.scalar.activation(o, src_ap, AF.Identity, bias=rsnm[:],
                                     scale=rs[:])

        # ---- setup: load weights, x0 (+ host-transposed copies).  The
        # warmup collective initializes the CC channel during setup so the
        # first real AllReduce runs at steady-state latency. ----
        with nc.named_scope("setup"):
            nc.vector.memset(wu_sb[:], 0.0)
            nc.sync.dma_start(wu_loc[:, :], wu_sb[:])
            nc.gpsimd.collective_compute(
                "AllReduce", ALU.add, replica_groups=RG,
                ins=[wu_loc[:, :]], outs=[wu_sum[:, :]])
            for d in range(nD):
                nc.sync.dma_start(xT_bf[0][d][:],
                                  x0T_d[d * P:(d + 1) * P, :])
                nc.sync.dma_start(wenc_sb[d][:], wenc_d[d * P:(d + 1) * P, :])
                nc.sync.dma_start(wencv_sb[d][:], wencv_d[d * P:(d + 1) * P, :])
                nc.sync.dma_start(wlm_sb[d][:], wlm_d[d * P:(d + 1) * P, :])
            for k in range(nK):
                nc.sync.dma_start(wdec_sb[k][:], wdec_d[k * P:(k + 1) * P, :])
            nc.sync.dma_start(ident_sb[:], ident_d[:, :])
            nc.sync.dma_start(maskt_sb[:], maskt_d[:, :])
            for t in range(nTB):
                nc.sync.dma_start(x_bf[0][t][:], x0b_d[t * P:(t + 1) * P, :])
                nc.sync.dma_start(x_f32[t][:], x0_d[t * P:(t + 1) * P, :])

        # x_sparse tiles from p1 are kept (2-chunk window, double parity)
        # and reused by p3's xy product -- no recompute.
        xs_store = {0: [None] * nK, 1: [None] * nK}

        def emit_p1_unit(l, ts, j, dve):
            """relu(enc^T x) + rope for pair-tile j of chunk (l, ts)."""
            xTl = xT_bf[l % 2]
            c0, c1 = ts * SUP, (ts + 1) * SUP
            ct = spT.tile([P, SUP], mm_dt, tag="ctc", name="ct")
            st = spT.tile([P, SUP], mm_dt, tag="stc", name="st")
            nc.sync.dma_start(ct[:], ctab_d[j * P:(j + 1) * P, c0:c1])
            nc.sync.dma_start(st[:], stab_d[j * P:(j + 1) * P, c0:c1])
            psA = ppb.tile([P, SUP], F32, tag="big", name="psA")
            psB = ppb.tile([P, SUP], F32, tag="big", name="psB")
            for d in range(nD):
                nc.tensor.matmul(psA[:], wenc_sb[d][:, j * P:(j + 1) * P],
                                 xTl[d][:, c0:c1],
                                 start=(d == 0), stop=(d == nD - 1))
            for d in range(nD):
                nc.tensor.matmul(psB[:],
                                 wenc_sb[d][:, (j + nJ) * P:(j + nJ + 1) * P],
                                 xTl[d][:, c0:c1],
                                 start=(d == 0), stop=(d == nD - 1))
            xsA = spX.tile([P, SUP], mm_dt, tag="xsr", name="xsA")
            xsB = spX.tile([P, SUP], mm_dt, tag="xsr", name="xsB")
            par = (l * nTS + ts) % 2
            xs_store[par][j] = xsA
            xs_store[par][j + nJ] = xsB
            # relus stay on the scalar engine: gpsimd cannot read PSUM
            nc.scalar.activation(xsA[:], psA[:], AF.Relu)
            nc.scalar.activation(xsB[:], psB[:], AF.Relu)
            if dve:
                eng, rtag = nc.vector, "rtv"
            else:
                eng, rtag = nc.gpsimd, "rtg"
            qa = qrT[j][:, c0:c1]
            qb = qrT[j + nJ][:, c0:c1]
            t1 = spB.tile([P, SUP], mm_dt, tag=rtag, name="t1", bufs=1)
            eng.tensor_tensor(qa, xsA[:], ct[:], ALU.mult)
            eng.tensor_tensor(t1[:], xsB[:], st[:], ALU.mult)
            eng.tensor_tensor(qa, qa, t1[:], ALU.subtract)
            t2 = spB.tile([P, SUP], mm_dt, tag=rtag, name="t2", bufs=1)
            eng.tensor_tensor(qb, xsB[:], ct[:], ALU.mult)
            eng.tensor_tensor(t2[:], xsA[:], st[:], ALU.mult)
            eng.tensor_tensor(qb, qb, t2[:], ALU.add)

        def emit_p4(l, ts):
            nxt = (l + 1) % 2
            xbn, xTn = x_bf[nxt], xT_bf[nxt]
            c0, c1 = ts * SUP, (ts + 1) * SUP
            with nc.named_scope(f"l{l}c{ts}_p4"):
                pin = spP.tile([P, nQ * D], F32, tag="pin", name="pin", bufs=1)
                nc.sync.dma_start(
                    pin[:].rearrange("p (n d) -> p n d", n=nQ),
                    p_sum[l][c0:c1, :].rearrange("(n p) d -> p n d", p=P))
                for q in range(nQ):
                    t = nQ * ts + q
                    xr = sp2.tile([P, D], F32, tag="lnr", name="xr", bufs=3)
                    nc.vector.tensor_tensor(xr[:], x_f32[t][:],
                                            pin[:, q * D:(q + 1) * D],
                                            ALU.add)
                    ln_chain(xr[:], [x_f32[t][:], xbn[t][:]])
                    for d in range(nD):
                        trp = pps.tile([P, P], mm_dt, tag="acc", name="trp4")
                        nc.tensor.transpose(
                            trp[:], xbn[t][:, d * P:(d + 1) * P],
                            ident_sb[:])
                        nc.vector.tensor_copy(
                            xTn[d][:, t * P:(t + 1) * P], trp[:])

        def emit_iteration(g):
            l, ts = divmod(g, nTS)
            par = l % 2
            xb = x_bf[par]
            xTl = xT_bf[par]
            c0, c1 = ts * SUP, (ts + 1) * SUP
            nsb = nQ * ts + nQ

            # prefetch target: chunk g+1 (maybe next layer).  Cross-layer
            # rope overwrites qrT cols this chunk's first nQ score units
            # still read as lhsT, so those units must be emitted first.
            if g + 1 < nG:
                l2, ts2 = divmod(g + 1, nTS)
                if l2 != l and not XLAYER_PREFETCH:
                    l2 = None
            else:
                l2 = None
            p1_off = nQ if (l2 is not None and l2 != l) else 0

            with nc.named_scope(f"l{l}c{ts}_p12"):
                if g == 0 or (ts == 0 and not XLAYER_PREFETCH):
                    for j in range(nJ):
                        emit_p1_unit(l, ts, j, dve=(j < max(1, nJ - 2)))

                ykv_ps = [pps.tile([P, D], F32, tag="acc", name=f"ykv{q}")
                          for q in range(nQ)]
                ykv_n = [None] * nQ
                pend = [None]

                def flush_ykv():
                    if pend[0] is None:
                        return
                    sb, stsb, q0 = pend[0]
                    for q in range(q0, nQ):
                        nc.tensor.matmul(
                            ykv_ps[q][:], stsb[:, q * P:(q + 1) * P],
                            xb[sb][:],
                            start=(sb == 0), stop=(sb == nQ * ts + q))
                    pend[0] = None
                    r = sb - nQ * ts
                    if r >= 0:
                        # ykv row-block r is complete; LN it now so the
                        # chain latency hides behind later score units.
                        ykv_n[r] = sp2.tile([P, D], mm_dt, tag="ykvn",
                                            name=f"ykvn{r}")
                        ln_chain(ykv_ps[r][:], [ykv_n[r][:]])

                for u in range(max(nsb, p1_off + ROPE_DVE_J)):
                    if u < nsb:
                        sb = u
                        r = sb - nQ * ts
                        q0 = max(0, r)
                        st_ps = ppb.tile([P, SUP], F32, tag="big",
                                         name="st_ps")
                        dst = st_ps[:, q0 * P:SUP]
                        for k in range(nK):
                            nc.tensor.matmul(
                                dst, qrT[k][:, sb * P:(sb + 1) * P],
                                qrT[k][:, c0 + q0 * P:c0 + SUP],
                                start=(k == 0), stop=(k == nK - 1))
                        flush_ykv()
                        st_sb = spB.tile([P, SUP], mm_dt, tag="stsb",
                                         name="st_sb", bufs=2)
                        if r >= 0:
                            nc.vector.tensor_tensor(
                                st_sb[:, r * P:(r + 1) * P],
                                st_ps[:, r * P:(r + 1) * P], maskt_sb[:],
                                ALU.mult)
                            if r + 1 < nQ:
                                nc.scalar.copy(st_sb[:, (r + 1) * P:SUP],
                                               st_ps[:, (r + 1) * P:SUP])
                        else:
                            nc.scalar.copy(st_sb[:], st_ps[:])
                        pend[0] = (sb, st_sb, q0)
                    if l2 is not None and p1_off <= u < p1_off + ROPE_DVE_J:
                        emit_p1_unit(l2, ts2, u - p1_off, dve=True)
                flush_ykv()

            # ---- p3: gpsimd p1 units + ykv transposes, then
            # y_sparse / xy / p using the saved x_sparse tiles ----
            with nc.named_scope(f"l{l}c{ts}_p3"):
                if l2 is not None:
                    for j in range(ROPE_DVE_J, nJ):
                        emit_p1_unit(l2, ts2, j, dve=False)
                xsr = xs_store[g % 2]

                for q in range(nQ):
                    tb = nQ * ts + q
                    for d in range(nD):
                        trp = pps.tile([P, P], mm_dt, tag="acc", name="trp2")
                        nc.tensor.transpose(trp[:],
                                            ykv_n[q][:, d * P:(d + 1) * P],
                                            ident_sb[:])
                        nc.vector.tensor_copy(
                            ykvT[d][:, tb * P:(tb + 1) * P], trp[:])

                YLAG = 3
                p_ps = [pps.tile([P, D], F32, tag="acc", name=f"pp{q}")
                        for q in range(nQ)]
                ys_ps = [None] * nK

                def emit_ys(k):
                    ys_ps[k] = ppb.tile([P, SUP], F32, tag="big", name="ys_ps")
                    for d in range(nD):
                        nc.tensor.matmul(
                            ys_ps[k][:], wencv_sb[d][:, k * P:(k + 1) * P],
                            ykvT[d][:, c0:c1],
                            start=(d == 0), stop=(d == nD - 1))

                for k in range(min(YLAG, nK)):
                    emit_ys(k)
                for k in range(nK):
                    ysr = spB.tile([P, SUP], mm_dt, tag="ysr", name="ysr", bufs=2)
                    nc.scalar.activation(ysr[:], ys_ps[k][:], AF.Relu)
                    xy = spB.tile([P, SUP], mm_dt, tag="xy", name="xy", bufs=2)
                    nc.vector.tensor_tensor(xy[:], ysr[:], xsr[k][:],
                                            ALU.mult)
                    for q in range(nQ):
                        nc.tensor.matmul(
                            p_ps[q][:], xy[:, q * P:(q + 1) * P],
                            wdec_sb[k][:],
                            start=(k == 0), stop=(k == nK - 1))
                    if k + YLAG < nK:
                        emit_ys(k + YLAG)
                pch = spP.tile([P, nQ * D], F32, tag="pch", name="pch", bufs=1)
                for q in range(nQ):
                    nc.scalar.copy(pch[:, q * D:(q + 1) * D], p_ps[q][:])

            # ---- AR: ship p rows, AllReduce this chunk ----
            with nc.named_scope(f"l{l}c{ts}_ar"):
                nc.sync.dma_start(
                    p_loc[l][c0:c1, :].rearrange("(n p) d -> p n d", p=P),
                    pch[:].rearrange("p (n d) -> p n d", n=nQ))
                nc.gpsimd.collective_compute(
                    "AllReduce", ALU.add, replica_groups=RG,
                    ins=[p_loc[l][c0:c1, :]], outs=[p_sum[l][c0:c1, :]])

            if g >= DEFER:
                emit_p4(*divmod(g - DEFER, nTS))

        for g in range(nG):
            emit_iteration(g)
        pending = [divmod(g, nTS) for g in range(nG - DEFER, nG)]

        # ---- final: out = x @ lm_head; pending p4s interleave so the
        # early lm_head chunks cover the last AllReduces ----
        with nc.named_scope("final"):
            if pending:
                emit_p4(*pending[0])
            xTf = xT_bf[NL % 2]
            for ts in range(nTS):
                if ts == nTS - 1:
                    for pc in pending[1:]:
                        emit_p4(*pc)
                och = spP.tile([P, nQ * V], F32, tag="och", name="och",
                               bufs=1)
                for q in range(nQ):
                    t = nQ * ts + q
                    o_ps = pps.tile([P, V], F32, tag="acc", name="o_ps")
                    for d in range(nD):
                        nc.tensor.matmul(o_ps[:],
                                         xTf[d][:, t * P:(t + 1) * P],
                                         wlm_sb[d][:],
                                         start=(d == 0), stop=(d == nD - 1))
                    nc.vector.tensor_copy(och[:, q * V:(q + 1) * V], o_ps[:])
                nc.sync.dma_start(
                    out_d[ts * SUP:(ts + 1) * SUP, :].rearrange(
                        "(n p) v -> p n v", p=P),
                    och[:].rearrange("p (n v) -> p n v", n=nQ))

        for cm in reversed(_cms):
            cm.__exit__(None, None, None)
        for f in reversed(_keep):
            f()
        _keep.clear()

    nc.compile()
    return nc


def host_inputs(idx, embed, encoder, encoder_v, decoder, lm_head, cfg,
                mm_dt=BF16):
    """Build the 8 per-core input maps (host-side prep is O(MB) copies)."""
    T, D, N, NL, V = cfg["T"], cfg["D"], cfg["N"], cfg["NL"], cfg["V"]
    NH, B = cfg["NH"], cfg["B"]
    np_mm = np.dtype(mybir.dt.np(mm_dt))

    idx = np.asarray(idx)
    embed = np.asarray(embed, dtype=np.float32)
    encoder = np.asarray(encoder, dtype=np.float32)
    encoder_v = np.asarray(encoder_v, dtype=np.float32)
    decoder = np.asarray(decoder, dtype=np.float32)
    lm_head = np.asarray(lm_head, dtype=np.float32)

    # initial x = ln(embed[idx]) in f32 (cheap: B*T*D)
    e = embed[idx]  # (B, T, D)
    mu = e.mean(-1, keepdims=True)
    var = ((e - mu) ** 2).mean(-1, keepdims=True)
    x0 = ((e - mu) / np.sqrt(var + EPS)).astype(np.float32)

    # rope tables in pair-permuted transposed layout [N/2, T]
    theta = np.float32(2.0 ** 16)
    q = (np.floor(np.arange(N, dtype=np.float32) / 2.0) * 2.0).astype(np.float32)
    freqs = (1.0 / (theta ** (q / np.float32(N))) /
             np.float32(2.0 * math.pi)).astype(np.float32)
    fp = freqs[0::2]  # (N/2,)
    ph = fp[:, None] * np.arange(T, dtype=np.float32)[None, :]
    pm = ((ph % np.float32(1.0)) * np.float32(2.0 * math.pi)).astype(np.float32)
    ctab = np.cos(pm).astype(np_mm)
    stab = np.sin(pm).astype(np_mm)

    perm = np.concatenate([np.arange(0, N, 2), np.arange(1, N, 2)])
    ident = np.eye(P, dtype=np_mm)
    maskt = np.triu(np.ones((P, P), np.float32), k=1).astype(np_mm)  # s < t

    x0b = x0.astype(np_mm)
    x0T = np.ascontiguousarray(x0b.transpose(0, 2, 1))

    in_maps = []
    for c in range(B * NH):
        b, h = c // NH, c % NH
        in_maps.append({
            "x0": x0[b],
            "x0b": x0b[b],
            "x0T": x0T[b],
            "wenc": encoder[h][:, perm].astype(np_mm),
            "wencv": encoder_v[h][:, perm].astype(np_mm),
            "wdec": decoder[h * N:(h + 1) * N, :][perm, :].astype(np_mm),
            "wlm": lm_head.astype(np_mm),
            "ctab": ctab,
            "stab": stab,
            "ident": ident,
            "maskt": maskt,
        })
    return in_maps


_NC_CACHE = {}


def _get_nc(cfg_key, cfg, mm_dt, n_cores):
    if cfg_key not in _NC_CACHE:
        _NC_CACHE[cfg_key] = build_nc(cfg, mm_dt=mm_dt, n_cores=n_cores)
    return _NC_CACHE[cfg_key]


def kernel(idx, embed, encoder, encoder_v, decoder, lm_head):
    cfg = FULL_CFG
    NH, B = cfg["NH"], cfg["B"]
    n_cores = B * NH
    in_maps = host_inputs(idx, embed, encoder, encoder_v, decoder, lm_head, cfg)
    nc = _get_nc("full_bf16", cfg, BF16, n_cores)
    res = run_bass_kernel_spmd(nc, in_maps, core_ids=list(range(n_cores)))
    out = np.stack([np.asarray(res.results[b * NH]["out"], dtype=np.float32)
                    for b in range(B)], axis=0)
    return out
